# revision 1
# baseline (speedup 1.0000x reference)
"""Trainium2 Bass kernel for nn_Conv2d_int8_est_T (LUT-based int8 quantized 3x3 conv).

Math notes:
  - The provided lut is the exact int8 product table lut[a+128,b+128] = a*b, so the
    LUT conv == integer conv.  Quantized values lie in [-128,127]; they are exact in
    bf16, and every partial sum is an integer < 2^24, so a bf16 matmul with fp32 PSUM
    accumulation reproduces the int32 accumulation bit-exactly.
  - Rounding (jnp.round == round-half-even) is done with the fp32 magic-number trick
    (+2^23*1.5, -2^23*1.5) on the vector engine, which rounds RNE.
  - Tf needs the global absmax of x; each core reduces its own batch shard and a
    scalar AllReduce(max) combines them.  Tw needs absmax of the (replicated) weight,
    computed locally on every core.

Sharding: data-parallel over batch (8 images -> 8 cores); weights/bias replicated.
"""

import sys

for _p in ("/opt/trn_rl_repo",):
    if _p not in sys.path:
        sys.path.insert(0, _p)

import numpy as np

B, CIN, COUT, H, W, KS = 8, 64, 128, 32, 32, 3
OH, OW = H, W
PW = 34          # padded row width (W + 2)
PADN = 1184      # padded image buffer columns (>= 34*34, rounded up)
MAGIC = 12582912.0  # 1.5 * 2^23: fp32 RNE rounding magic constant

N_CORES = 8
_USE_COLLECTIVE = True
_STOP_AFTER = 'full'

# Offset blocks: (lo_offset, hi_offset) pairs sharing one K=128 matmul via the
# shifted duplicate (hi[p] = lo[p+1]), plus three leftover K=64 singles.  The
# singles all read the lo half: mixing lo-half and hi-half K=64 LDWEIGHTS in one
# PSUM accumulation group crashes the runtime (found by bisection).
# Offsets d=(ki,kj); flat position in padded row-major = ki*34+kj.
PAIR_BLOCKS = [((0, 0), (0, 1)), ((1, 1), (1, 2)), ((2, 0), (2, 1))]
SOLO_BLOCKS = [(0, 2), (1, 0), (2, 2)]  # K=64 matmuls, weights in rows 0:64

_cache = {}

W_COLS = (len(PAIR_BLOCKS) + len(SOLO_BLOCKS)) * 128  # 768


def _pack_weights(weight):
    """[COUT,CIN,3,3] f32 -> [128, 768] f32; pair blocks use both row halves,
    solo blocks use rows 0:64 (rows 64:128 stay zero)."""
    wp = np.zeros((128, W_COLS), np.float32)
    for b, (lo, hi) in enumerate(PAIR_BLOCKS):
        wp[0:64, b * 128:(b + 1) * 128] = weight[:, :, lo[0], lo[1]].T
        wp[64:128, b * 128:(b + 1) * 128] = weight[:, :, hi[0], hi[1]].T
    for j, d in enumerate(SOLO_BLOCKS):
        c = (3 + j) * 128
        wp[0:64, c:c + 128] = weight[:, :, d[0], d[1]].T
    return np.ascontiguousarray(wp)


def _build(trace):
    import concourse.bass as bass
    import concourse.bacc as bacc
    import concourse.mybir as mybir
    import concourse.tile as tile

    f32 = mybir.dt.float32
    bf16 = mybir.dt.bfloat16
    Alu = mybir.AluOpType

    nc = bacc.Bacc(num_devices=N_CORES)

    x_d = nc.dram_tensor("x", [CIN, OH * OW], f32, kind="ExternalInput")
    w_d = nc.dram_tensor("w", [128, W_COLS], f32, kind="ExternalInput")
    bias_d = nc.dram_tensor("bias", [COUT, 1], f32, kind="ExternalInput")
    tf0_d = nc.dram_tensor("tf0", [1, 1], f32, kind="ExternalInput")
    tw0_d = nc.dram_tensor("tw0", [1, 1], f32, kind="ExternalInput")
    gmax_d = nc.dram_tensor("gmax", [1, 1], f32, kind="ExternalInput")
    out_d = nc.dram_tensor("out", [COUT, OH * OW], f32, kind="ExternalOutput")

    idn_d = nc.inline_tensor(np.eye(128, dtype=np.float32), name="idn")

    with tile.TileContext(nc) as tc:
        with (
            tc.tile_pool(name="sbuf", bufs=1) as sb,
            tc.tile_pool(name="psum", bufs=1, space="PSUM") as ps,
            tc.tile_pool(name="dram", bufs=1, space="DRAM") as dr,
        ):
            # ---- input DMAs ----
            xin = sb.tile([128, OH * OW], f32, name="xin")
            nc.sync.dma_start(xin[0:64, :], x_d[:])
            nc.sync.dma_start(xin[64:128, :], x_d[:])
            wsb = sb.tile([128, W_COLS], f32, name="wsb")
            nc.sync.dma_start(wsb[:], w_d[:])
            bias_t = sb.tile([128, 1], f32, name="bias_t")
            nc.sync.dma_start(bias_t[:], bias_d[:])
            tf0 = sb.tile([1, 1], f32, name="tf0")
            nc.sync.dma_start(tf0[:], tf0_d[:])
            tw0 = sb.tile([1, 1], f32, name="tw0")
            nc.sync.dma_start(tw0[:], tw0_d[:])
            idn = sb.tile([128, 128], f32, name="idn_t")
            nc.sync.dma_start(idn[:], idn_d[:])

            # ---- constants ----
            c127 = sb.tile([1, 1], f32, name="c127")
            nc.vector.memset(c127[:], 127.0)
            ones = sb.tile([1, 128], f32, name="ones")
            nc.vector.memset(ones[:], 1.0)
            xq = sb.tile([128, PADN], bf16, name="xq")
            nc.vector.memset(xq[:], 0.0)

            # ---- global |x| max: computed by the absmax launch + host max of
            # the 8 per-core partials (the "all-reduce" is one scalar) ----
            gmax = sb.tile([1, 1], f32, name="gmax")
            nc.sync.dma_start(gmax[:], gmax_d[:])

            # ---- w absmax (local; weight fully replicated) ----
            wr = sb.tile([128, 1], f32, name="wr")
            nc.vector.tensor_reduce(
                wr[:], wsb[:], axis=mybir.AxisListType.X, op=Alu.max,
                apply_absolute_value=True,
            )
            wr_t = ps.tile([1, 128], f32, name="wr_t")
            nc.tensor.transpose(wr_t[:], wr[:], idn[:])
            wmax = sb.tile([1, 1], f32, name="wmax")
            nc.vector.tensor_reduce(
                wmax[:], wr_t[:], axis=mybir.AxisListType.X, op=Alu.max,
            )

            # ---- thresholds & scales (scalars, fp32, matching jax op order) ----
            R127 = float(np.float32(1.0) / np.float32(127.0))

            def ema(name, t0, m):
                a = sb.tile([1, 1], f32, name=name + "_a")
                nc.vector.tensor_scalar_mul(a[:], t0[:], 0.95)
                b_ = sb.tile([1, 1], f32, name=name + "_b")
                nc.vector.tensor_scalar_mul(b_[:], m[:], 0.05)
                t = sb.tile([1, 1], f32, name=name)
                nc.vector.tensor_tensor(t[:], a[:], b_[:], op=Alu.add)
                return t

            def div127(name, bt):
                """q = RN(127 / bt), bit-exact (reciprocal + Dekker TwoProd +
                Markstein correction; exhaustively verified over fp32 [0.5,8))."""
                s = sb.tile([1, 16], f32, name=name + "_s", tag=name + "_s")
                r, q0, cq, qh, ql = s[:, 0:1], s[:, 1:2], s[:, 2:3], s[:, 3:4], s[:, 4:5]
                cb, bh, bl, p = s[:, 5:6], s[:, 6:7], s[:, 7:8], s[:, 8:9]
                t0, t1, err, e0, e = s[:, 9:10], s[:, 10:11], s[:, 11:12], s[:, 12:13], s[:, 13:14]
                nc.vector.reciprocal(r, bt[:])
                nc.vector.tensor_scalar_mul(q0, r, 127.0)
                nc.vector.tensor_scalar_mul(cq, q0, 4097.0)
                nc.vector.tensor_tensor(t0, cq, q0, op=Alu.subtract)
                nc.vector.tensor_tensor(qh, cq, t0, op=Alu.subtract)
                nc.vector.tensor_tensor(ql, q0, qh, op=Alu.subtract)
                nc.vector.tensor_scalar_mul(cb, bt[:], 4097.0)
                nc.vector.tensor_tensor(t0, cb, bt[:], op=Alu.subtract)
                nc.vector.tensor_tensor(bh, cb, t0, op=Alu.subtract)
                nc.vector.tensor_tensor(bl, bt[:], bh, op=Alu.subtract)
                nc.vector.tensor_tensor(p, q0, bt[:], op=Alu.mult)
                nc.vector.tensor_tensor(t0, qh, bh, op=Alu.mult)
                nc.vector.tensor_tensor(err, t0, p, op=Alu.subtract)
                nc.vector.tensor_tensor(t1, qh, bl, op=Alu.mult)
                nc.vector.tensor_tensor(err, err, t1, op=Alu.add)
                nc.vector.tensor_tensor(t1, ql, bh, op=Alu.mult)
                nc.vector.tensor_tensor(err, err, t1, op=Alu.add)
                nc.vector.tensor_tensor(t1, ql, bl, op=Alu.mult)
                nc.vector.tensor_tensor(err, err, t1, op=Alu.add)
                nc.vector.tensor_tensor(e0, c127[:], p, op=Alu.subtract)
                nc.vector.tensor_tensor(e, e0, err, op=Alu.subtract)
                nc.vector.tensor_tensor(t1, e, r, op=Alu.mult)
                q = sb.tile([1, 1], f32, name=name)
                nc.vector.tensor_tensor(q[:], q0, t1, op=Alu.add)
                return q

            Tw = ema("Tw", tw0, wmax)
            scal_w = sb.tile([1, 2], f32, name="scal_w")
            qw = div127("qw", Tw)
            nc.vector.tensor_copy(scal_w[:, 0:1], qw[:])
            nc.vector.tensor_scalar_mul(scal_w[:, 1:2], Tw[:], R127)
            bw = ps.tile([128, 2], f32, name="bw")
            nc.tensor.matmul(bw[:], ones[:], scal_w[:], start=True, stop=True)
            wsc = sb.tile([128, 2], f32, name="wsc")
            nc.vector.tensor_copy(wsc[:], bw[:])

            Tf = ema("Tf", tf0, gmax)
            scal_f = sb.tile([1, 2], f32, name="scal_f")
            qf = div127("qf", Tf)
            nc.vector.tensor_copy(scal_f[:, 0:1], qf[:])
            nc.vector.tensor_scalar_mul(scal_f[:, 1:2], Tf[:], R127)
            bf_ = ps.tile([128, 2], f32, name="bf_")
            nc.tensor.matmul(bf_[:], ones[:], scal_f[:], start=True, stop=True)
            fsc = sb.tile([128, 2], f32, name="fsc")
            nc.vector.tensor_copy(fsc[:], bf_[:])
            # epilogue scale s ~= (Tf/127)*(Tw/127) (continuous path; ulp-level
            # deviation from the reference is within the fp32 envelope)
            sep = sb.tile([128, 1], f32, name="sep")
            nc.vector.tensor_tensor(sep[:], fsc[:, 1:2], wsc[:, 1:2], op=Alu.mult)

            # ---- quantize w -> bf16 [128, 640] ----
            # mult and magic-add must be separate instructions: the reference
            # rounds fl(w*scale) before round-to-int, so a fused single-rounding
            # FMA would diverge on boundary values.
            wq1 = sb.tile([128, W_COLS], f32, name="wq1")
            nc.vector.tensor_scalar_mul(wq1[:], wsb[:], wsc[:, 0:1])
            nc.vector.tensor_scalar_add(wq1[:], wq1[:], MAGIC)
            nc.vector.tensor_scalar(
                wq1[:], wq1[:], MAGIC, -128.0, op0=Alu.subtract, op1=Alu.max,
            )
            wqb = sb.tile([128, W_COLS], bf16, name="wqb")
            nc.vector.tensor_scalar(wqb[:], wq1[:], 127.0, None, op0=Alu.min)

            # ---- quantize x -> padded bf16 image (lo) + left-shifted copy (hi) ----
            xq1 = sb.tile([128, OH * OW], f32, name="xq1")
            nc.vector.tensor_scalar_mul(xq1[:], xin[:], fsc[:, 0:1])
            nc.vector.tensor_scalar_add(xq1[:], xq1[:], MAGIC)
            nc.vector.tensor_scalar(
                xq1[:], xq1[:], MAGIC, -128.0, op0=Alu.subtract, op1=Alu.max,
            )

            def interior(part_lo, part_hi, off):
                sl = xq[part_lo:part_hi, off:off + 32 * PW]
                return sl.rearrange("p (r c) -> p r c", c=PW)[:, :, 0:32]

            src_lo = xq1[0:64, :].rearrange("p (r c) -> p r c", c=32)
            src_hi = xq1[64:128, :].rearrange("p (r c) -> p r c", c=32)
            nc.vector.tensor_scalar(
                interior(0, 64, PW + 1), src_lo, 127.0, None, op0=Alu.min,
            )
            nc.vector.tensor_scalar(
                interior(64, 128, PW), src_hi, 127.0, None, op0=Alu.min,
            )

            # ---- conv: 2 spatial halves x 6 matmuls accumulating in PSUM ----
            def win(part_lo, part_hi, off):
                sl = xq[part_lo:part_hi, off:off + 16 * PW]
                return sl.rearrange("p (r c) -> p r c", c=PW)[:, :, 0:32]

            out_sb = sb.tile([128, OH * OW], f32, name="out_sb")
            for st in range(2):
                r0 = st * 16
                acc = ps.tile([128, 512], f32, name=f"acc{st}", tag=f"acc{st}")
                for b, (lo, _hi) in enumerate(PAIR_BLOCKS):
                    nc.tensor.matmul(
                        acc[:],
                        wqb[:, b * 128:(b + 1) * 128],
                        win(0, 128, (r0 + lo[0]) * PW + lo[1]),
                        start=(b == 0), stop=False,
                    )
                for j, d in enumerate(SOLO_BLOCKS):
                    c = (3 + j) * 128
                    nc.tensor.matmul(
                        acc[:], wqb[0:64, c:c + 128],
                        win(0, 64, (r0 + d[0]) * PW + d[1]),
                        start=False, stop=(j == len(SOLO_BLOCKS) - 1),
                    )
                # epilogue: out = acc * s + bias
                nc.vector.tensor_scalar(
                    out_sb[:, st * 512:(st + 1) * 512], acc[:],
                    sep[:], bias_t[:], op0=Alu.mult, op1=Alu.add,
                )

            nc.sync.dma_start(out_d[:], out_sb[:])

    nc.compile()
    return nc


def _build_absmax():
    import concourse.bacc as bacc
    import concourse.mybir as mybir
    import concourse.tile as tile

    f32 = mybir.dt.float32
    Alu = mybir.AluOpType

    nc = bacc.Bacc(num_devices=N_CORES)
    x_d = nc.dram_tensor("x", [CIN, OH * OW], f32, kind="ExternalInput")
    m_d = nc.dram_tensor("m", [CIN, 1], f32, kind="ExternalOutput")
    with tile.TileContext(nc) as tc:
        with tc.tile_pool(name="sbuf", bufs=2) as sb:
            t = sb.tile([CIN, OH * OW], f32, name="t")
            nc.sync.dma_start(t[:], x_d[:])
            r = sb.tile([CIN, 1], f32, name="r")
            nc.vector.tensor_reduce(
                r[:], t[:], axis=mybir.AxisListType.X, op=Alu.max,
                apply_absolute_value=True,
            )
            nc.sync.dma_start(m_d[:], r[:])
    nc.compile()
    return nc


def _install_ntff_shim():
    import types
    try:
        from antenv.axon_hooks import get_axon_ntff_profile_hook  # noqa: F401
        return
    except ImportError:
        pass
    try:
        from trn_agent_boot.trn_boot import _ntff_profile_via_ctypes
        hook = _ntff_profile_via_ctypes("/opt/axon/libaxon_pjrt.so")
    except Exception:
        hook = None
    mod = types.ModuleType("antenv.axon_hooks")
    mod._hook = hook
    mod.get_axon_ntff_profile_hook = lambda: mod._hook
    mod.set_axon_ntff_profile_hook = lambda h: setattr(mod, "_hook", h)
    sys.modules["antenv.axon_hooks"] = mod


def run(inputs, trace=False):
    """Run the kernel; returns (output [8,128,32,32] f32, (resA, resB))."""
    from concourse import bass_utils

    if trace:
        _install_ntff_shim()

    if "nc_a" not in _cache:
        _cache["nc_a"] = _build_absmax()
    if "nc_b" not in _cache:
        _cache["nc_b"] = _build(trace)
    nc_a, nc_b = _cache["nc_a"], _cache["nc_b"]

    x = np.asarray(inputs["x"], np.float32)
    weight = np.asarray(inputs["weight"], np.float32)
    bias = np.asarray(inputs["bias"], np.float32).reshape(COUT, 1)
    tf0 = np.asarray(inputs["T_feature"], np.float32).reshape(1, 1)
    tw0 = np.asarray(inputs["T_weight"], np.float32).reshape(1, 1)

    x_shards = [np.ascontiguousarray(x[i].reshape(CIN, OH * OW))
                for i in range(N_CORES)]

    # launch A: per-core per-partition |x| maxes
    res_a = bass_utils.run_bass_kernel_spmd(
        nc_a, [{"x": xs} for xs in x_shards],
        core_ids=list(range(N_CORES)), trace=trace,
    )
    # scalar all-reduce: fp32 max is order-independent, so this equals
    # jnp.max(jnp.abs(x)) bitwise
    gmax = np.float32(max(res_a.results[i]["m"].max() for i in range(N_CORES)))
    gmax = np.asarray(gmax, np.float32).reshape(1, 1)

    wp = _pack_weights(weight)
    in_maps = []
    for i in range(N_CORES):
        in_maps.append({
            "x": x_shards[i],
            "w": wp,
            "bias": bias,
            "tf0": tf0,
            "tw0": tw0,
            "gmax": gmax,
        })

    res_b = bass_utils.run_bass_kernel_spmd(
        nc_b, in_maps, core_ids=list(range(N_CORES)), trace=trace,
    )
    out = np.stack(
        [res_b.results[i]["out"].reshape(COUT, OH, OW) for i in range(N_CORES)]
    ).astype(np.float32)
    return out, (res_a, res_b)


def kernel(x, weight, bias, lut, gradient_lut, T_feature, T_weight):
    out, _ = run({
        "x": x, "weight": weight, "bias": bias, "lut": lut,
        "gradient_lut": gradient_lut, "T_feature": T_feature,
        "T_weight": T_weight,
    })
    return out



# revision 5
# speedup vs baseline: 1.5174x; 1.5174x over previous
"""Trainium2 Bass kernel for nn_Conv2d_int8_est_T (LUT-based int8 quantized 3x3 conv).

Math notes:
  - The provided lut is the exact int8 product table lut[a+128,b+128] = a*b, so the
    LUT conv == integer conv.  Quantized values lie in [-128,127]; they are exact in
    bf16, and every partial sum is an integer < 2^24, so a bf16 matmul with fp32 PSUM
    accumulation reproduces the int32 accumulation bit-exactly.
  - Rounding (round-half-even) via the fp32 magic-number trick on the vector engine.
  - Tf needs the global absmax of x.  Instead of a second launch or a collective
    (both ~20us of latency), every core redundantly scans a bf16 copy of the full x
    (1.1 MB) shipped alongside its own shard.  The bf16 rounding of the max changes
    Tf by <=2^-9 relative, which perturbs the quantized conv by ~1e-3 relative --
    far inside the 2e-2 gate.
  - Host pre-pads the core's own image (and its column-shifted duplicate for the
    pair-matmul trick) into the final [128, 1184] bf16 layout, so the device does no
    memsets / strided pad copies: quantization maps padding zeros to zeros.

Sharding: data-parallel over batch (8 images -> 8 cores); weights/bias replicated.
"""

import sys

for _p in ("/opt/trn_rl_repo",):
    if _p not in sys.path:
        sys.path.insert(0, _p)

import numpy as np
import ml_dtypes

BF16 = ml_dtypes.bfloat16

B, CIN, COUT, H, W, KS = 8, 64, 128, 32, 32, 3
OH, OW = H, W
PW = 34          # padded row width (W + 2)
PADN = 1184      # padded image buffer columns (>= 34*34, rounded up)
OTHW = B * 512   # all shards for the absmax scan, each [64,1024] viewed as [128,512]
MAGIC = 12582912.0     # 1.5 * 2^23: fp32 RNE rounding magic constant

N_CORES = 8

# Offset blocks: (lo_offset, hi_offset) pairs sharing one K=128 matmul via the
# shifted duplicate (hi[p] = lo[p+1]), plus three leftover K=64 singles.  The
# singles all read the lo half: mixing lo-half and hi-half K=64 LDWEIGHTS in one
# PSUM accumulation group crashes the runtime (found by bisection).
PAIR_BLOCKS = [((0, 0), (0, 1)), ((1, 1), (1, 2)), ((2, 0), (2, 1))]
SOLO_BLOCKS = [(0, 2), (1, 0), (2, 2)]  # K=64 matmuls, weights in rows 0:64

_cache = {}

W_COLS = (len(PAIR_BLOCKS) + len(SOLO_BLOCKS)) * 128  # 768
WALL_COLS = W_COLS + 3  # + tf0, tw0, bias columns


def _pack_weights(weight):
    """[COUT,CIN,3,3] f32 -> [128, 768] f32; pair blocks use both row halves,
    solo blocks use rows 0:64 (rows 64:128 stay zero)."""
    wp = np.zeros((128, W_COLS), np.float32)
    for b, (lo, hi) in enumerate(PAIR_BLOCKS):
        wp[0:64, b * 128:(b + 1) * 128] = weight[:, :, lo[0], lo[1]].T
        wp[64:128, b * 128:(b + 1) * 128] = weight[:, :, hi[0], hi[1]].T
    for j, d in enumerate(SOLO_BLOCKS):
        c = (3 + j) * 128
        wp[0:64, c:c + 128] = weight[:, :, d[0], d[1]].T
    return np.ascontiguousarray(wp)


def _build():
    import concourse.bacc as bacc
    import concourse.bass_isa as bass_isa
    import concourse.mybir as mybir
    import concourse.tile as tile

    f32 = mybir.dt.float32
    bf16 = mybir.dt.bfloat16
    Alu = mybir.AluOpType
    X = mybir.AxisListType.X

    nc = bacc.Bacc(num_devices=N_CORES)

    wall_d = nc.dram_tensor("wall", [128, WALL_COLS], f32, kind="ExternalInput")
    xpad_d = nc.dram_tensor("xpad", [128, PADN], bf16, kind="ExternalInput")
    xoth_d = nc.dram_tensor("xoth", [128, OTHW], bf16, kind="ExternalInput")
    out_d = nc.dram_tensor("out", [COUT, OH * OW], f32, kind="ExternalOutput")

    R127 = float(np.float32(1.0) / np.float32(127.0))

    with tile.TileContext(nc) as tc:
        with (
            tc.tile_pool(name="sbuf", bufs=1) as sb,
            tc.tile_pool(name="psum", bufs=1, space="PSUM") as ps,
        ):
            # ---- input DMAs (sync HWDGE ring) ----
            wall = sb.tile([128, WALL_COLS], f32, name="wall")
            nc.sync.dma_start(wall[:], wall_d[:])
            xpad = sb.tile([128, PADN], bf16, name="xpad")
            nc.sync.dma_start(xpad[:], xpad_d[:])
            xoth = sb.tile([128, OTHW], bf16, name="xoth")
            nc.sync.dma_start(xoth[:], xoth_d[:])

            # ---- absmax partials (vector): w first (lands first) ----
            partials = sb.tile([128, 2], f32, name="partials")
            nc.vector.tensor_reduce(
                partials[:, 1:2], wall[:, 0:W_COLS], axis=X, op=Alu.max,
                apply_absolute_value=True,
            )
            # full x (all 8 shards): max and min fold chains; absmax at the end
            # (abs_max ALU op is not supported by the backend)
            sc = sb.tile([128, 2048], bf16, name="sc")
            sc2 = sb.tile([128, 2048], bf16, name="sc2")
            nc.vector.tensor_tensor(
                sc[:], xoth[:, 0:2048], xoth[:, 2048:4096], op=Alu.max)
            nc.vector.tensor_tensor(
                sc2[:], xoth[:, 0:2048], xoth[:, 2048:4096], op=Alu.min)
            nc.vector.tensor_tensor(
                sc[:, 0:1024], sc[:, 0:1024], sc[:, 1024:2048], op=Alu.max)
            nc.vector.tensor_tensor(
                sc2[:, 0:1024], sc2[:, 0:1024], sc2[:, 1024:2048], op=Alu.min)
            nc.vector.tensor_tensor(
                sc[:, 0:512], sc[:, 0:512], sc[:, 512:1024], op=Alu.max)
            nc.vector.tensor_tensor(
                sc2[:, 0:512], sc2[:, 0:512], sc2[:, 512:1024], op=Alu.min)
            rmax = sb.tile([128, 2], f32, name="rmax")
            nc.vector.tensor_reduce(
                rmax[:, 0:1], sc[:, 0:512], axis=X, op=Alu.max)
            nc.vector.tensor_reduce(
                rmax[:, 1:2], sc2[:, 0:512], axis=X, op=Alu.min)
            # |x|max partial = max(maxpart, -minpart), fused negate+max
            nc.vector.tensor_scalar(
                partials[:, 0:1], rmax[:, 1:2], -1.0, rmax[:, 0:1],
                op0=Alu.mult, op1=Alu.max,
            )

            # ---- cross-partition max + broadcast in one gpsimd op ----
            m = sb.tile([128, 2], f32, name="m")
            nc.gpsimd.partition_all_reduce(
                m[:], partials[:], channels=128,
                reduce_op=bass_isa.ReduceOp.max,
            )

            # ---- thresholds & scales: T = 0.95*t0 + 0.05*m (cols: x, w) ----
            e1 = sb.tile([128, 2], f32, name="e1")
            nc.vector.tensor_scalar_mul(e1[:], wall[:, W_COLS:W_COLS + 2], 0.95)
            T = sb.tile([128, 2], f32, name="T")
            nc.vector.tensor_scalar_mul(T[:], m[:], 0.05)
            nc.vector.tensor_tensor(T[:], T[:], e1[:], op=Alu.add)
            r = sb.tile([128, 2], f32, name="r")
            nc.vector.reciprocal(r[:], T[:])
            q = sb.tile([128, 2], f32, name="q")
            nc.vector.tensor_scalar_mul(q[:], r[:], 127.0)
            s = sb.tile([128, 2], f32, name="s")
            nc.vector.tensor_scalar_mul(s[:], T[:], R127)
            sep = sb.tile([128, 1], f32, name="sep")
            nc.vector.tensor_tensor(sep[:], s[:, 0:1], s[:, 1:2], op=Alu.mult)

            # ---- quantize w -> bf16 [128, 768] ----
            wq1 = sb.tile([128, W_COLS], f32, name="wq1")
            nc.vector.tensor_scalar(
                wq1[:], wall[:, 0:W_COLS], q[:, 1:2], MAGIC,
                op0=Alu.mult, op1=Alu.add,
            )
            nc.vector.tensor_scalar(
                wq1[:], wq1[:], MAGIC, -128.0, op0=Alu.subtract, op1=Alu.max,
            )
            wqb = sb.tile([128, W_COLS], bf16, name="wqb")
            nc.vector.tensor_scalar(wqb[:], wq1[:], 127.0, None, op0=Alu.min)

            # ---- quantize x (padding zeros stay zero) -> bf16 [128, 1184] ----
            xq1 = sb.tile([128, PADN], f32, name="xq1")
            nc.vector.tensor_scalar(
                xq1[:], xpad[:], q[:, 0:1], MAGIC, op0=Alu.mult, op1=Alu.add,
            )
            nc.vector.tensor_scalar(
                xq1[:], xq1[:], MAGIC, -128.0, op0=Alu.subtract, op1=Alu.max,
            )
            xqb = sb.tile([128, PADN], bf16, name="xqb")
            nc.vector.tensor_scalar(xqb[:], xq1[:], 127.0, None, op0=Alu.min)

            # ---- conv: 2 spatial halves x 6 matmuls accumulating in PSUM ----
            def win(part_lo, part_hi, off):
                sl = xqb[part_lo:part_hi, off:off + 16 * PW]
                return sl.rearrange("p (r c) -> p r c", c=PW)[:, :, 0:32]

            out_sb = sb.tile([128, OH * OW], f32, name="out_sb")
            for st in range(2):
                r0 = st * 16
                acc = ps.tile([128, 512], f32, name=f"acc{st}", tag=f"acc{st}")
                for b, (lo, _hi) in enumerate(PAIR_BLOCKS):
                    nc.tensor.matmul(
                        acc[:],
                        wqb[:, b * 128:(b + 1) * 128],
                        win(0, 128, (r0 + lo[0]) * PW + lo[1]),
                        start=(b == 0), stop=False,
                    )
                for j, d in enumerate(SOLO_BLOCKS):
                    c = (3 + j) * 128
                    nc.tensor.matmul(
                        acc[:], wqb[0:64, c:c + 128],
                        win(0, 64, (r0 + d[0]) * PW + d[1]),
                        start=False, stop=(j == len(SOLO_BLOCKS) - 1),
                    )
                # epilogue: out = acc * s + bias
                nc.vector.tensor_scalar(
                    out_sb[:, st * 512:(st + 1) * 512], acc[:],
                    sep[:], wall[:, W_COLS + 2:W_COLS + 3],
                    op0=Alu.mult, op1=Alu.add,
                )
                # output DMA on the Activation HWDGE ring (parallel issue path)
                nc.scalar.dma_start(
                    out_d[:, st * 512:(st + 1) * 512],
                    out_sb[:, st * 512:(st + 1) * 512],
                )

    nc.compile()
    return nc


def _install_ntff_shim():
    import types
    try:
        from antenv.axon_hooks import get_axon_ntff_profile_hook  # noqa: F401
        return
    except ImportError:
        pass
    try:
        from trn_agent_boot.trn_boot import _ntff_profile_via_ctypes
        hook = _ntff_profile_via_ctypes("/opt/axon/libaxon_pjrt.so")
    except Exception:
        hook = None
    mod = types.ModuleType("antenv.axon_hooks")
    mod._hook = hook
    mod.get_axon_ntff_profile_hook = lambda: mod._hook
    mod.set_axon_ntff_profile_hook = lambda h: setattr(mod, "_hook", h)
    sys.modules["antenv.axon_hooks"] = mod


def _pack_inputs(inputs):
    x = np.asarray(inputs["x"], np.float32)
    weight = np.asarray(inputs["weight"], np.float32)
    bias = np.asarray(inputs["bias"], np.float32)
    tf0 = float(np.asarray(inputs["T_feature"], np.float32).reshape(-1)[0])
    tw0 = float(np.asarray(inputs["T_weight"], np.float32).reshape(-1)[0])

    wall = np.zeros((128, WALL_COLS), np.float32)
    wall[:, 0:W_COLS] = _pack_weights(weight)
    wall[:, W_COLS] = tf0
    wall[:, W_COLS + 1] = tw0
    wall[:, W_COLS + 2] = bias

    xb = x.astype(BF16)  # [8,64,32,32]
    lo = np.zeros((B, CIN, PW, PW), BF16)
    lo[:, :, 1:33, 1:33] = xb
    hi = np.zeros((B, CIN, PW, PW), BF16)
    hi[:, :, 1:33, 0:32] = xb
    xpad_all = np.zeros((B, 128, PADN), BF16)
    xpad_all[:, 0:64, :PW * PW] = lo.reshape(B, CIN, PW * PW)
    xpad_all[:, 64:128, :PW * PW] = hi.reshape(B, CIN, PW * PW)

    # all 8 shards (signed bf16) for the absmax scan, identical on every core
    xoth = np.ascontiguousarray(
        xb.reshape(B, 128, 512).transpose(1, 0, 2).reshape(128, B * 512))
    in_maps = []
    for i in range(N_CORES):
        in_maps.append({
            "wall": wall,
            "xpad": np.ascontiguousarray(xpad_all[i]),
            "xoth": xoth,
        })
    return in_maps


def run(inputs, trace=False):
    """Run the kernel; returns (output [8,128,32,32] f32, (res,))."""
    from concourse import bass_utils

    if trace:
        _install_ntff_shim()

    if "nc" not in _cache:
        _cache["nc"] = _build()
    nc = _cache["nc"]

    in_maps = _pack_inputs(inputs)
    res = bass_utils.run_bass_kernel_spmd(
        nc, in_maps, core_ids=list(range(N_CORES)), trace=trace,
    )
    out = np.stack(
        [res.results[i]["out"].reshape(COUT, OH, OW) for i in range(N_CORES)]
    ).astype(np.float32)
    return out, (res,)


def kernel(x, weight, bias, lut, gradient_lut, T_feature, T_weight):
    out, _ = run({
        "x": x, "weight": weight, "bias": bias, "lut": lut,
        "gradient_lut": gradient_lut, "T_feature": T_feature,
        "T_weight": T_weight,
    })
    return out
